# revision 3
# baseline (speedup 1.0000x reference)
"""Bass/Tile TRN2 kernel for nn_AttentionLayer (B=2, S=2048, D=1024, H=16).

Sharding: 8 cores = 2 (batch) x 4 (head groups of 4 heads each).
Each core computes Q/K/V projections for its 256 output columns and
full attention for its 4 heads; host concatenates the per-core
[S, 256] output slices.

Device-side layout choices:
  - Host pre-transposes q/k/v to x^T [D, S] so projections contract D on
    the partition dim with no on-device transposes.
  - Q^T, K^T produced head-transposed [e, s]; V produced natural [s, e]
    with a fused all-ones column per head (denominator rides the PV
    matmul as output row 64).
  - scores^T = K Q^T per head; softmax exp on ScalarE from PSUM (scale
    1/8 fused); no max-subtraction (scores are O(10), fp32 exp safe).
  - PV: out^T[h d+1, sq] = V'^T E accumulated over sk chunks in PSUM.
  - PE transpose of out^T -> out, then normalize by the ones-row sum.
  - All matmuls in float32r (TF32-like, 1 cycle/row at N>=256).
"""

import sys

sys.path.insert(0, "/opt/trn_rl_repo")

import numpy as np

import concourse.bacc as bacc
import concourse.mybir as mybir
from concourse.masks import make_identity
from concourse.tile import TileContext
from concourse.bass_utils import run_bass_kernel_spmd

F32 = mybir.dt.float32
F32R = mybir.dt.float32r
AF = mybir.ActivationFunctionType
ALU = mybir.AluOpType

B, S, D, H = 2, 2048, 1024, 16
HD = D // H            # 64
NCORES = 8
HPC = 4                # heads per core
E = HPC * HD           # 256 output cols per core
EV = HPC * (HD + 1)    # 260: V' with ones column per head
DCH = D // 128         # 8 d chunks
ST = S // 512          # 4 s tiles (projections)
SQT = S // 1024        # 2 sq tiles (attention)
SKC = S // 128         # 16 sk chunks
SCALE = 1.0 / np.sqrt(HD)


def build_kernel(repeat: int = 1):
    nc = bacc.Bacc()
    xqT = nc.dram_tensor("xqT", [D, S], F32, kind="ExternalInput")
    xkT = nc.dram_tensor("xkT", [D, S], F32, kind="ExternalInput")
    xvT = nc.dram_tensor("xvT", [D, S], F32, kind="ExternalInput")
    wq = nc.dram_tensor("wq", [D, E], F32, kind="ExternalInput")
    wk = nc.dram_tensor("wk", [D, E], F32, kind="ExternalInput")
    wv = nc.dram_tensor("wv", [D, EV], F32, kind="ExternalInput")
    bq = nc.dram_tensor("bq", [128, 2], F32, kind="ExternalInput")
    bk = nc.dram_tensor("bk", [128, 2], F32, kind="ExternalInput")
    bv = nc.dram_tensor("bv", [128, EV], F32, kind="ExternalInput")
    out = nc.dram_tensor("out", [S, E], F32, kind="ExternalOutput")

    with TileContext(nc) as tc:
        with tc.tile_pool(name="wsb", bufs=1) as wsb, \
             tc.tile_pool(name="xsb", bufs=3) as xsb, \
             tc.tile_pool(name="qkv", bufs=1) as qkv, \
             tc.tile_pool(name="esb", bufs=3) as esb, \
             tc.tile_pool(name="osb", bufs=4) as osb:

            # ---- constants / weights (loaded once) ----
            wq_t = wsb.tile([128, DCH, E], F32R)
            wk_t = wsb.tile([128, DCH, E], F32R)
            wv_t = wsb.tile([128, DCH, EV], F32R)
            nc.gpsimd.dma_start(wq_t[:], wq.rearrange("(c p) e -> p c e", p=128))
            nc.gpsimd.dma_start(wk_t[:], wk.rearrange("(c p) e -> p c e", p=128))
            nc.gpsimd.dma_start(wv_t[:], wv.rearrange("(c p) e -> p c e", p=128))
            bq_t = wsb.tile([128, 2], F32)
            bk_t = wsb.tile([128, 2], F32)
            bv_t = wsb.tile([128, EV], F32)
            nc.sync.dma_start(bq_t[:], bq[:])
            nc.sync.dma_start(bk_t[:], bk[:])
            nc.sync.dma_start(bv_t[:], bv[:])
            ident = wsb.tile([65, 65], F32)
            make_identity(nc, ident[:])

            for _ in range(repeat):
                # persistent per-iteration products
                QT_t = qkv.tile([128, 2, S], F32R, tag="QT")   # [e%128, epair, s]
                KT_t = qkv.tile([128, 2, S], F32R, tag="KT")
                V_t = qkv.tile([128, SKC, EV], F32R, tag="V")  # [s%128, schunk, 4*(hd+1)]

                # ---------------- projections ----------------
                with tc.tile_pool(name="pps", bufs=4, space="PSUM") as pps:
                    for si in range(ST):
                        sl = slice(512 * si, 512 * (si + 1))
                        xq_t = xsb.tile([128, DCH, 512], F32R, tag="x")
                        nc.gpsimd.dma_start(
                            xq_t[:], xqT[:, sl].rearrange("(c p) s -> p c s", p=128))
                        xk_t = xsb.tile([128, DCH, 512], F32R, tag="x")
                        nc.gpsimd.dma_start(
                            xk_t[:], xkT[:, sl].rearrange("(c p) s -> p c s", p=128))
                        for (x_t, w_t, b_t, o_t) in ((xq_t, wq_t, bq_t, QT_t),
                                                     (xk_t, wk_t, bk_t, KT_t)):
                            for et in range(2):
                                ps = pps.tile([128, 512], F32, tag="pj")
                                for c in range(DCH):
                                    nc.tensor.matmul(
                                        ps[:], w_t[:, c, 128 * et:128 * (et + 1)],
                                        x_t[:, c], start=(c == 0), stop=(c == DCH - 1))
                                nc.vector.tensor_scalar(
                                    out=o_t[:, et, sl], in0=ps[:],
                                    scalar1=b_t[:, et:et + 1], scalar2=None,
                                    op0=ALU.add)
                        xv_t = xsb.tile([128, DCH, 512], F32R, tag="x")
                        nc.gpsimd.dma_start(
                            xv_t[:], xvT[:, sl].rearrange("(c p) s -> p c s", p=128))
                        for k in range(4):
                            psv = pps.tile([128, EV], F32, tag="pj")
                            for c in range(DCH):
                                nc.tensor.matmul(
                                    psv[:], xv_t[:, c, 128 * k:128 * (k + 1)],
                                    wv_t[:, c], start=(c == 0), stop=(c == DCH - 1))
                            nc.vector.tensor_tensor(
                                out=V_t[:, 4 * si + k, :], in0=psv[:], in1=bv_t[:],
                                op=ALU.add)

                # ---------------- attention ----------------
                with tc.tile_pool(name="stp", bufs=2, space="PSUM") as stp, \
                     tc.tile_pool(name="pvp", bufs=2, space="PSUM") as pvp:
                    for pr in range(2):          # head pairs
                        for sqt in range(SQT):   # sq tiles of 1024
                            sq0 = 1024 * sqt
                            pv_a = pvp.tile([65, 1024], F32, tag="pv", name="pv_a")
                            pv_b = pvp.tile([65, 1024], F32, tag="pv", name="pv_b")
                            pv_ps = [pv_a, pv_b]
                            for ck in range(SKC):
                                for h in range(2):   # head within pair
                                    hh = 2 * pr + h
                                    hp = slice(64 * h, 64 * (h + 1))
                                    st = stp.tile([128, 1024], F32, tag="st")
                                    for q in range(2):
                                        nc.tensor.matmul(
                                            st[:, 512 * q:512 * (q + 1)],
                                            KT_t[hp, pr, 128 * ck:128 * (ck + 1)],
                                            QT_t[hp, pr,
                                                 sq0 + 512 * q:sq0 + 512 * (q + 1)],
                                            start=True, stop=True)
                                    e_t = esb.tile([128, 1024], F32R)
                                    nc.scalar.activation(e_t[:], st[:], AF.Exp,
                                                         scale=float(SCALE))
                                    for q in range(2):
                                        nc.tensor.matmul(
                                            pv_ps[h][:, 512 * q:512 * (q + 1)],
                                            V_t[:, ck, 65 * hh:65 * hh + 65],
                                            e_t[:, 512 * q:512 * (q + 1)],
                                            start=(ck == 0), stop=(ck == SKC - 1))
                            # drain pv: transpose + normalize + store
                            for h in range(2):
                                hh = 2 * pr + h
                                ov = osb.tile([65, 1024], F32, tag="ov")
                                nc.vector.tensor_copy(ov[:], pv_ps[h][:])
                                for k in range(8):
                                    ot = stp.tile([128, 65], F32, tag="st")
                                    nc.tensor.transpose(
                                        ot[:], ov[:, 128 * k:128 * (k + 1)], ident[:])
                                    rc = osb.tile([128, 1], F32, tag="rc")
                                    nc.vector.reciprocal(rc[:], ot[:, 64:65])
                                    ob = osb.tile([128, HD], F32, tag="ob")
                                    nc.vector.tensor_scalar(
                                        out=ob[:], in0=ot[:, 0:HD], scalar1=rc[:],
                                        scalar2=None, op0=ALU.mult)
                                    r0 = sq0 + 128 * k
                                    nc.sync.dma_start(
                                        out[r0:r0 + 128, HD * hh:HD * (hh + 1)],
                                        ob[:])
    nc.compile()
    return nc


_NC_CACHE = {}


def _get_nc(repeat: int = 1):
    if repeat not in _NC_CACHE:
        _NC_CACHE[repeat] = build_kernel(repeat)
    return _NC_CACHE[repeat]


def _shard_inputs(q, k, v, Wq, bq, Wk, bk, Wv, bv):
    """Build the 8 per-core input maps (host-side marshaling)."""
    xT = {}
    for b in range(B):
        xT[("q", b)] = np.ascontiguousarray(np.asarray(q)[b].T)
        xT[("k", b)] = np.ascontiguousarray(np.asarray(k)[b].T)
        xT[("v", b)] = np.ascontiguousarray(np.asarray(v)[b].T)
    Wq, Wk, Wv = (np.asarray(a, np.float32) for a in (Wq, Wk, Wv))
    bq, bk, bv = (np.asarray(a, np.float32) for a in (bq, bk, bv))
    in_maps = []
    for c in range(NCORES):
        b, g = divmod(c, HPC)
        sl = slice(E * g, E * (g + 1))
        wv_p = np.zeros((D, EV), np.float32)
        bv_p = np.zeros((128, EV), np.float32)
        for h in range(HPC):
            wv_p[:, 65 * h:65 * h + HD] = Wv[:, E * g + HD * h:E * g + HD * (h + 1)]
            bv_p[:, 65 * h:65 * h + HD] = bv[E * g + HD * h:E * g + HD * (h + 1)]
            bv_p[:, 65 * h + HD] = 1.0
        in_maps.append({
            "xqT": xT[("q", b)], "xkT": xT[("k", b)], "xvT": xT[("v", b)],
            "wq": np.ascontiguousarray(Wq[:, sl]),
            "wk": np.ascontiguousarray(Wk[:, sl]),
            "wv": wv_p,
            "bq": np.ascontiguousarray(bq[sl].reshape(2, 128).T),
            "bk": np.ascontiguousarray(bk[sl].reshape(2, 128).T),
            "bv": bv_p,
        })
    return in_maps


def kernel(q, k, v, Wq, bq, Wk, bk, Wv, bv):
    nc = _get_nc()
    in_maps = _shard_inputs(q, k, v, Wq, bq, Wk, bk, Wv, bv)
    res = run_bass_kernel_spmd(nc, in_maps, core_ids=list(range(NCORES)))
    outp = np.empty((B, S, D), np.float32)
    for c in range(NCORES):
        b, g = divmod(c, HPC)
        outp[b, :, E * g:E * (g + 1)] = res.results[c]["out"]
    return outp


# revision 4
# speedup vs baseline: 1.1262x; 1.1262x over previous
"""Bass/Tile TRN2 kernel for nn_AttentionLayer (B=2, S=2048, D=1024, H=16).

Sharding: 8 cores = 2 (batch) x 4 (head groups of 4 heads each).
Each core computes Q/K/V projections for its 256 output columns and
full attention for its 4 heads; host concatenates the per-core
[S, 256] output slices.

Device-side layout choices:
  - Host pre-transposes q/k/v to x^T [D, S] so projections contract D on
    the partition dim with no on-device transposes.
  - Q^T, K^T produced head-transposed [e, s]; V produced natural [s, e]
    with a fused all-ones column per head (denominator rides the PV
    matmul as output row 64).
  - scores^T = K Q^T per head; softmax exp on ScalarE from PSUM (scale
    1/8 fused); no max-subtraction (scores are O(10), fp32 exp safe).
  - PV: out^T[h d+1, sq] = V'^T E accumulated over sk chunks in PSUM.
  - PE transpose of out^T -> out, then normalize by the ones-row sum.
  - All matmuls in float32r (TF32-like, 1 cycle/row at N>=256).
"""

import sys

sys.path.insert(0, "/opt/trn_rl_repo")

import numpy as np

import concourse.bacc as bacc
import concourse.mybir as mybir
from concourse.masks import make_identity
from concourse.tile import TileContext
from concourse.bass_utils import run_bass_kernel_spmd

F32 = mybir.dt.float32
F32R = mybir.dt.float32r
AF = mybir.ActivationFunctionType
ALU = mybir.AluOpType

B, S, D, H = 2, 2048, 1024, 16
HD = D // H            # 64
NCORES = 8
HPC = 4                # heads per core
E = HPC * HD           # 256 output cols per core
EV = HPC * (HD + 1)    # 260: V' with ones column per head
DCH = D // 128         # 8 d chunks
ST = S // 512          # 4 s tiles (projections)
SQT = S // 1024        # 2 sq tiles (attention)
SKC = S // 128         # 16 sk chunks
SCALE = 1.0 / np.sqrt(HD)


def build_kernel(repeat: int = 1):
    nc = bacc.Bacc()
    xqT = nc.dram_tensor("xqT", [D, S], F32, kind="ExternalInput")
    xkT = nc.dram_tensor("xkT", [D, S], F32, kind="ExternalInput")
    xvT = nc.dram_tensor("xvT", [D, S], F32, kind="ExternalInput")
    wq = nc.dram_tensor("wq", [D, E], F32, kind="ExternalInput")
    wk = nc.dram_tensor("wk", [D, E], F32, kind="ExternalInput")
    wv = nc.dram_tensor("wv", [D, EV], F32, kind="ExternalInput")
    bq = nc.dram_tensor("bq", [128, 2], F32, kind="ExternalInput")
    bk = nc.dram_tensor("bk", [128, 2], F32, kind="ExternalInput")
    bv = nc.dram_tensor("bv", [128, EV], F32, kind="ExternalInput")
    out = nc.dram_tensor("out", [S, E], F32, kind="ExternalOutput")

    with TileContext(nc) as tc:
        with tc.tile_pool(name="wsb", bufs=1) as wsb, \
             tc.tile_pool(name="xsb", bufs=3) as xsb, \
             tc.tile_pool(name="qkv", bufs=1) as qkv, \
             tc.tile_pool(name="esb", bufs=3) as esb, \
             tc.tile_pool(name="osb", bufs=4) as osb, \
             tc.tile_pool(name="pps", bufs=2, space="PSUM") as pps, \
             tc.tile_pool(name="stp", bufs=2, space="PSUM") as stp, \
             tc.tile_pool(name="pvp", bufs=2, space="PSUM") as pvp:

            # ---- constants / weights (loaded once) ----
            wq_t = wsb.tile([128, DCH, E], F32R)
            wk_t = wsb.tile([128, DCH, E], F32R)
            wv_t = wsb.tile([128, DCH, EV], F32R)
            nc.gpsimd.dma_start(wq_t[:], wq.rearrange("(c p) e -> p c e", p=128))
            nc.gpsimd.dma_start(wk_t[:], wk.rearrange("(c p) e -> p c e", p=128))
            nc.gpsimd.dma_start(wv_t[:], wv.rearrange("(c p) e -> p c e", p=128))
            bq_t = wsb.tile([128, 2], F32)
            bk_t = wsb.tile([128, 2], F32)
            bv_t = wsb.tile([128, EV], F32)
            nc.sync.dma_start(bq_t[:], bq[:])
            nc.sync.dma_start(bk_t[:], bk[:])
            nc.sync.dma_start(bv_t[:], bv[:])
            ident = wsb.tile([65, 65], F32)
            make_identity(nc, ident[:])
            # touch Exp early so the ACT table load happens during projections
            warm = wsb.tile([128, 1], F32)
            nc.scalar.activation(warm[:], bq_t[:, 0:1], AF.Exp)

            def load_x(src, si):
                sl = slice(512 * si, 512 * (si + 1))
                x_t = xsb.tile([128, DCH, 512], F32R, tag="x", name=f"x_{si}")
                nc.gpsimd.dma_start(
                    x_t[:], src[:, sl].rearrange("(c p) s -> p c s", p=128))
                return x_t

            def project_qk(x_t, w_t, b_t, o_t, si, ets=(0, 1)):
                sl = slice(512 * si, 512 * (si + 1))
                for et in ets:
                    ps = pps.tile([128, 512], F32, tag="pj", name="ps_qk")
                    for c in range(DCH):
                        nc.tensor.matmul(
                            ps[:], w_t[:, c, 128 * et:128 * (et + 1)],
                            x_t[:, c], start=(c == 0), stop=(c == DCH - 1))
                    nc.vector.tensor_scalar(
                        out=o_t[:, et, sl], in0=ps[:],
                        scalar1=b_t[:, et:et + 1], scalar2=None, op0=ALU.add)

            def project_v(x_t, si):
                for k in range(4):
                    psv = pps.tile([128, EV], F32, tag="pj", name="ps_v")
                    for c in range(DCH):
                        nc.tensor.matmul(
                            psv[:], x_t[:, c, 128 * k:128 * (k + 1)],
                            wv_t[:, c], start=(c == 0), stop=(c == DCH - 1))
                    nc.vector.tensor_tensor(
                        out=V_t[:, 4 * si + k, :], in0=psv[:], in1=bv_t[:],
                        op=ALU.add)

            def attention_block(pr, sqt):
                """One head pair x one 512-wide sq tile."""
                sq0 = 512 * sqt
                pv_a = pvp.tile([65, 512], F32, tag="pv", name="pv_a")
                pv_b = pvp.tile([65, 512], F32, tag="pv", name="pv_b")
                pv_ps = (pv_a, pv_b)
                for cp in range(SKC // 2):     # sk chunk pairs
                    for h in range(2):         # head within pair
                        hh = 2 * pr + h
                        hp = slice(64 * h, 64 * (h + 1))
                        st = stp.tile([128, 1024], F32, tag="st", name="st")
                        for q in range(2):     # the two sk chunks of the pair
                            ck = 2 * cp + q
                            nc.tensor.matmul(
                                st[:, 512 * q:512 * (q + 1)],
                                KT_t[hp, pr, 128 * ck:128 * (ck + 1)],
                                QT_t[hp, pr, sq0:sq0 + 512],
                                start=True, stop=True)
                        e_t = esb.tile([128, 1024], F32R, name="e_t")
                        nc.scalar.activation(e_t[:], st[:], AF.Exp,
                                             scale=float(SCALE))
                        for q in range(2):
                            ck = 2 * cp + q
                            nc.tensor.matmul(
                                pv_ps[h][:],
                                V_t[:, ck, 65 * hh:65 * hh + 65],
                                e_t[:, 512 * q:512 * (q + 1)],
                                start=(cp == 0 and q == 0),
                                stop=(cp == SKC // 2 - 1 and q == 1))
                # drain pv: transpose + normalize + store
                for h in range(2):
                    hh = 2 * pr + h
                    ov = osb.tile([65, 512], F32, tag="ov", name="ov")
                    nc.vector.tensor_copy(ov[:], pv_ps[h][:])
                    for k in range(4):
                        ot = pps.tile([128, 65], F32, tag="pj", name="ot")
                        nc.tensor.transpose(
                            ot[:], ov[:, 128 * k:128 * (k + 1)], ident[:])
                        rc = osb.tile([128, 1], F32, tag="rc", name="rc")
                        nc.vector.reciprocal(rc[:], ot[:, 64:65])
                        ob = osb.tile([128, HD], F32, tag="ob", name="ob")
                        nc.vector.tensor_scalar(
                            out=ob[:], in0=ot[:, 0:HD], scalar1=rc[:],
                            scalar2=None, op0=ALU.mult)
                        r0 = sq0 + 128 * k
                        nc.sync.dma_start(
                            out[r0:r0 + 128, HD * hh:HD * (hh + 1)], ob[:])

            for _ in range(repeat):
                # persistent per-iteration products
                QT_t = qkv.tile([128, 2, S], F32R, tag="QT", name="QT_t")
                KT_t = qkv.tile([128, 2, S], F32R, tag="KT", name="KT_t")
                V_t = qkv.tile([128, SKC, EV], F32R, tag="V", name="V_t")

                # K and V projections first (attention needs them in full),
                # then Q one s-tile at a time, chased by its attention blocks.
                for si in range(ST):
                    xk_t = load_x(xkT, si)
                    project_qk(xk_t, wk_t, bk_t, KT_t, si)
                for si in range(ST):
                    xv_t = load_x(xvT, si)
                    project_v(xv_t, si)
                for si in range(ST):
                    xq_t = load_x(xqT, si)
                    project_qk(xq_t, wq_t, bq_t, QT_t, si)
                    for pr in range(2):
                        attention_block(pr, si)
    nc.compile()
    return nc


_NC_CACHE = {}


def _get_nc(repeat: int = 1):
    if repeat not in _NC_CACHE:
        _NC_CACHE[repeat] = build_kernel(repeat)
    return _NC_CACHE[repeat]


def _shard_inputs(q, k, v, Wq, bq, Wk, bk, Wv, bv):
    """Build the 8 per-core input maps (host-side marshaling)."""
    xT = {}
    for b in range(B):
        xT[("q", b)] = np.ascontiguousarray(np.asarray(q)[b].T)
        xT[("k", b)] = np.ascontiguousarray(np.asarray(k)[b].T)
        xT[("v", b)] = np.ascontiguousarray(np.asarray(v)[b].T)
    Wq, Wk, Wv = (np.asarray(a, np.float32) for a in (Wq, Wk, Wv))
    bq, bk, bv = (np.asarray(a, np.float32) for a in (bq, bk, bv))
    in_maps = []
    for c in range(NCORES):
        b, g = divmod(c, HPC)
        sl = slice(E * g, E * (g + 1))
        wv_p = np.zeros((D, EV), np.float32)
        bv_p = np.zeros((128, EV), np.float32)
        for h in range(HPC):
            wv_p[:, 65 * h:65 * h + HD] = Wv[:, E * g + HD * h:E * g + HD * (h + 1)]
            bv_p[:, 65 * h:65 * h + HD] = bv[E * g + HD * h:E * g + HD * (h + 1)]
            bv_p[:, 65 * h + HD] = 1.0
        in_maps.append({
            "xqT": xT[("q", b)], "xkT": xT[("k", b)], "xvT": xT[("v", b)],
            "wq": np.ascontiguousarray(Wq[:, sl]),
            "wk": np.ascontiguousarray(Wk[:, sl]),
            "wv": wv_p,
            "bq": np.ascontiguousarray(bq[sl].reshape(2, 128).T),
            "bk": np.ascontiguousarray(bk[sl].reshape(2, 128).T),
            "bv": bv_p,
        })
    return in_maps


def kernel(q, k, v, Wq, bq, Wk, bk, Wv, bv):
    nc = _get_nc()
    in_maps = _shard_inputs(q, k, v, Wq, bq, Wk, bk, Wv, bv)
    res = run_bass_kernel_spmd(nc, in_maps, core_ids=list(range(NCORES)))
    outp = np.empty((B, S, D), np.float32)
    for c in range(NCORES):
        b, g = divmod(c, HPC)
        outp[b, :, E * g:E * (g + 1)] = res.results[c]["out"]
    return outp


# revision 6
# speedup vs baseline: 1.2414x; 1.1024x over previous
"""Bass/Tile TRN2 kernel for nn_AttentionLayer (B=2, S=2048, D=1024, H=16).

Sharding: 8 cores = 2 (batch) x 4 (head groups of 4 heads each).
Each core computes Q/K/V projections for its 256 output columns and
full attention for its 4 heads; host concatenates the per-core
[S, 256] output slices.

Device-side layout choices:
  - Host pre-transposes q/k/v to x^T [D, S] so projections contract D on
    the partition dim with no on-device transposes.
  - Q^T, K^T produced head-transposed [e, s]; V produced natural [s, e]
    with a fused all-ones column per head (denominator rides the PV
    matmul as output row 64).
  - scores^T = K Q^T per head; softmax exp on ScalarE from PSUM (scale
    1/8 fused); no max-subtraction (scores are O(10), fp32 exp safe).
  - PV: out^T[h d+1, sq] = V'^T E accumulated over sk chunks in PSUM.
  - PE transpose of out^T -> out, then normalize by the ones-row sum.
  - All matmuls in float32r (TF32-like, 1 cycle/row at N>=256).
"""

import sys

sys.path.insert(0, "/opt/trn_rl_repo")

import numpy as np

import concourse.bacc as bacc
import concourse.mybir as mybir
from concourse.masks import make_identity
from concourse.tile import TileContext
from concourse.bass_utils import run_bass_kernel_spmd

F32 = mybir.dt.float32
F32R = mybir.dt.float32r
AF = mybir.ActivationFunctionType
ALU = mybir.AluOpType

B, S, D, H = 2, 2048, 1024, 16
HD = D // H            # 64
NCORES = 8
HPC = 4                # heads per core
E = HPC * HD           # 256 output cols per core
EV = HPC * (HD + 1)    # 260: V' with ones column per head
DCH = D // 128         # 8 d chunks
ST = S // 512          # 4 s tiles (projections)
SQT = S // 1024        # 2 sq tiles (attention)
SKC = S // 128         # 16 sk chunks
SCALE = 1.0 / np.sqrt(HD)


def build_kernel(repeat: int = 1):
    nc = bacc.Bacc()
    xqT = nc.dram_tensor("xqT", [D, S], F32, kind="ExternalInput")
    xkT = nc.dram_tensor("xkT", [D, S], F32, kind="ExternalInput")
    xvT = nc.dram_tensor("xvT", [D, S], F32, kind="ExternalInput")
    wq = nc.dram_tensor("wq", [D, E], F32, kind="ExternalInput")
    wk = nc.dram_tensor("wk", [D, E], F32, kind="ExternalInput")
    wv = nc.dram_tensor("wv", [D, EV], F32, kind="ExternalInput")
    bq = nc.dram_tensor("bq", [128, 2], F32, kind="ExternalInput")
    bk = nc.dram_tensor("bk", [128, 2], F32, kind="ExternalInput")
    bv = nc.dram_tensor("bv", [128, EV], F32, kind="ExternalInput")
    out = nc.dram_tensor("out", [S, E], F32, kind="ExternalOutput")

    with TileContext(nc) as tc:
        with tc.tile_pool(name="wsb", bufs=1) as wsb, \
             tc.tile_pool(name="xsb", bufs=3) as xsb, \
             tc.tile_pool(name="qkv", bufs=1) as qkv, \
             tc.tile_pool(name="esb", bufs=3) as esb, \
             tc.tile_pool(name="osb", bufs=4) as osb, \
             tc.tile_pool(name="pps", bufs=2, space="PSUM") as pps, \
             tc.tile_pool(name="stp", bufs=2, space="PSUM") as stp, \
             tc.tile_pool(name="pvp", bufs=2, space="PSUM") as pvp:

            # ---- constants / weights (loaded once) ----
            wq_t = wsb.tile([128, DCH, E], F32R)
            wk_t = wsb.tile([128, DCH, E], F32R)
            wv_t = wsb.tile([128, DCH, EV], F32R)
            nc.gpsimd.dma_start(wq_t[:], wq.rearrange("(c p) e -> p c e", p=128))
            nc.gpsimd.dma_start(wk_t[:], wk.rearrange("(c p) e -> p c e", p=128))
            nc.gpsimd.dma_start(wv_t[:], wv.rearrange("(c p) e -> p c e", p=128))
            bq_t = wsb.tile([128, 2], F32)
            bk_t = wsb.tile([128, 2], F32)
            bv_t = wsb.tile([128, EV], F32)
            nc.sync.dma_start(bq_t[:], bq[:])
            nc.sync.dma_start(bk_t[:], bk[:])
            nc.sync.dma_start(bv_t[:], bv[:])
            ident = wsb.tile([65, 65], F32)
            make_identity(nc, ident[:])
            # touch Exp early so the ACT table load happens during projections
            warm = wsb.tile([128, 1], F32)
            nc.scalar.activation(warm[:], bq_t[:, 0:1], AF.Exp)

            def load_x(src, si):
                sl = slice(512 * si, 512 * (si + 1))
                x_t = xsb.tile([128, DCH, 512], F32R, tag="x", name=f"x_{si}")
                nc.gpsimd.dma_start(
                    x_t[:], src[:, sl].rearrange("(c p) s -> p c s", p=128))
                return x_t

            def project_qk(x_t, w_t, b_t, o_t, si, ets=(0, 1)):
                sl = slice(512 * si, 512 * (si + 1))
                for et in ets:
                    ps = pps.tile([128, 512], F32, tag="pj", name="ps_qk")
                    for c in range(DCH):
                        nc.tensor.matmul(
                            ps[:], w_t[:, c, 128 * et:128 * (et + 1)],
                            x_t[:, c], start=(c == 0), stop=(c == DCH - 1))
                    nc.vector.tensor_scalar(
                        out=o_t[:, et, sl], in0=ps[:],
                        scalar1=b_t[:, et:et + 1], scalar2=None, op0=ALU.add)

            def project_v(x_t, si):
                for k in range(4):
                    psv = pps.tile([128, EV], F32, tag="pj", name="ps_v")
                    for c in range(DCH):
                        nc.tensor.matmul(
                            psv[:], x_t[:, c, 128 * k:128 * (k + 1)],
                            wv_t[:, c], start=(c == 0), stop=(c == DCH - 1))
                    nc.vector.tensor_tensor(
                        out=V_t[:, 4 * si + k, :], in0=psv[:], in1=bv_t[:],
                        op=ALU.add)

            def attention_piece(ov_acc, pr, sqt, si):
                """Chunks 4si..4si+3 of the (pr, sqt) block.

                PV partials land in a transient PSUM tile per chunk pair and
                are accumulated into ov_acc[h] (SBUF) on the DVE, so only one
                pv PSUM slot is held at a time and many blocks can be in
                flight chunk-chasing the K/V loads.
                """
                sq0 = 512 * sqt
                for cpl in range(2):           # chunk pairs within the piece
                    for h in range(2):         # head within pair
                        hh = 2 * pr + h
                        hp = slice(64 * h, 64 * (h + 1))
                        st = stp.tile([128, 1024], F32, tag="st", name="st")
                        for q in range(2):
                            ck = 4 * si + 2 * cpl + q
                            nc.tensor.matmul(
                                st[:, 512 * q:512 * (q + 1)],
                                KT_t[hp, pr, 128 * ck:128 * (ck + 1)],
                                QT_t[hp, pr, sq0:sq0 + 512],
                                start=True, stop=True)
                        e_t = esb.tile([128, 1024], F32R, name="e_t")
                        nc.scalar.activation(e_t[:], st[:], AF.Exp,
                                             scale=float(SCALE))
                        pv = pvp.tile([65, 512], F32, tag="pv", name="pv")
                        for q in range(2):
                            ck = 4 * si + 2 * cpl + q
                            nc.tensor.matmul(
                                pv[:],
                                V_t[:, ck, 65 * hh:65 * hh + 65],
                                e_t[:, 512 * q:512 * (q + 1)],
                                start=(q == 0), stop=(q == 1))
                        if si == 0 and cpl == 0:
                            nc.vector.tensor_copy(ov_acc[h][:], pv[:])
                        else:
                            nc.vector.tensor_tensor(
                                out=ov_acc[h][:], in0=ov_acc[h][:], in1=pv[:],
                                op=ALU.add)

            def attention_drain(ov_acc, pr, sqt):
                """Transpose + normalize + store the (pr, sqt) block."""
                sq0 = 512 * sqt
                for h in range(2):
                    hh = 2 * pr + h
                    for k in range(4):
                        ot = pps.tile([128, 65], F32, tag="pj", name="ot")
                        nc.tensor.transpose(
                            ot[:], ov_acc[h][:, 128 * k:128 * (k + 1)], ident[:])
                        rc = osb.tile([128, 1], F32, tag="rc", name="rc")
                        nc.vector.reciprocal(rc[:], ot[:, 64:65])
                        ob = osb.tile([128, HD], F32, tag="ob", name="ob")
                        nc.vector.tensor_scalar(
                            out=ob[:], in0=ot[:, 0:HD], scalar1=rc[:],
                            scalar2=None, op0=ALU.mult)
                        r0 = sq0 + 128 * k
                        nc.sync.dma_start(
                            out[r0:r0 + 128, HD * hh:HD * (hh + 1)], ob[:])

            def new_block(pr, sqt):
                a = osb.tile([65, 512], F32, tag="ov", bufs=12, name=f"ova{pr}{sqt}")
                b = osb.tile([65, 512], F32, tag="ov", bufs=12, name=f"ovb{pr}{sqt}")
                return (a, b)

            for _ in range(repeat):
                # persistent per-iteration products
                QT_t = qkv.tile([128, 2, S], F32R, tag="QT", name="QT_t")
                KT_t = qkv.tile([128, 2, S], F32R, tag="KT", name="KT_t")
                V_t = qkv.tile([128, SKC, EV], F32R, tag="V", name="V_t")

                ov = {}
                # si=0 data first, then attention pieces chunk-chase the
                # remaining K/V (+Q) loads.
                xk = load_x(xkT, 0)
                project_qk(xk, wk_t, bk_t, KT_t, 0)
                xq = load_x(xqT, 0)
                project_qk(xq, wq_t, bq_t, QT_t, 0)
                xv = load_x(xvT, 0)
                project_v(xv, 0)
                for pr in range(2):
                    ov[(pr, 0)] = new_block(pr, 0)
                    attention_piece(ov[(pr, 0)], pr, 0, 0)

                xk = load_x(xkT, 1)
                project_qk(xk, wk_t, bk_t, KT_t, 1)
                xv = load_x(xvT, 1)
                project_v(xv, 1)
                for pr in range(2):
                    attention_piece(ov[(pr, 0)], pr, 0, 1)
                xq = load_x(xqT, 1)
                project_qk(xq, wq_t, bq_t, QT_t, 1)
                for pr in range(2):
                    ov[(pr, 1)] = new_block(pr, 1)
                    attention_piece(ov[(pr, 1)], pr, 1, 0)
                    attention_piece(ov[(pr, 1)], pr, 1, 1)

                xk = load_x(xkT, 2)
                project_qk(xk, wk_t, bk_t, KT_t, 2)
                xv = load_x(xvT, 2)
                project_v(xv, 2)
                for pr in range(2):
                    attention_piece(ov[(pr, 0)], pr, 0, 2)
                    attention_piece(ov[(pr, 1)], pr, 1, 2)

                xk = load_x(xkT, 3)
                project_qk(xk, wk_t, bk_t, KT_t, 3)
                xv = load_x(xvT, 3)
                project_v(xv, 3)
                for pr in range(2):
                    attention_piece(ov[(pr, 0)], pr, 0, 3)
                    attention_drain(ov[(pr, 0)], pr, 0)
                    attention_piece(ov[(pr, 1)], pr, 1, 3)
                    attention_drain(ov[(pr, 1)], pr, 1)

                for sqt in range(2, ST):
                    xq = load_x(xqT, sqt)
                    project_qk(xq, wq_t, bq_t, QT_t, sqt)
                    for pr in range(2):
                        ov[(pr, sqt)] = new_block(pr, sqt)
                        for si in range(ST):
                            attention_piece(ov[(pr, sqt)], pr, sqt, si)
                        attention_drain(ov[(pr, sqt)], pr, sqt)
    nc.compile()
    return nc


_NC_CACHE = {}


def _get_nc(repeat: int = 1):
    if repeat not in _NC_CACHE:
        _NC_CACHE[repeat] = build_kernel(repeat)
    return _NC_CACHE[repeat]


def _shard_inputs(q, k, v, Wq, bq, Wk, bk, Wv, bv):
    """Build the 8 per-core input maps (host-side marshaling)."""
    xT = {}
    for b in range(B):
        xT[("q", b)] = np.ascontiguousarray(np.asarray(q)[b].T)
        xT[("k", b)] = np.ascontiguousarray(np.asarray(k)[b].T)
        xT[("v", b)] = np.ascontiguousarray(np.asarray(v)[b].T)
    Wq, Wk, Wv = (np.asarray(a, np.float32) for a in (Wq, Wk, Wv))
    bq, bk, bv = (np.asarray(a, np.float32) for a in (bq, bk, bv))
    in_maps = []
    for c in range(NCORES):
        b, g = divmod(c, HPC)
        sl = slice(E * g, E * (g + 1))
        wv_p = np.zeros((D, EV), np.float32)
        bv_p = np.zeros((128, EV), np.float32)
        for h in range(HPC):
            wv_p[:, 65 * h:65 * h + HD] = Wv[:, E * g + HD * h:E * g + HD * (h + 1)]
            bv_p[:, 65 * h:65 * h + HD] = bv[E * g + HD * h:E * g + HD * (h + 1)]
            bv_p[:, 65 * h + HD] = 1.0
        in_maps.append({
            "xqT": xT[("q", b)], "xkT": xT[("k", b)], "xvT": xT[("v", b)],
            "wq": np.ascontiguousarray(Wq[:, sl]),
            "wk": np.ascontiguousarray(Wk[:, sl]),
            "wv": wv_p,
            "bq": np.ascontiguousarray(bq[sl].reshape(2, 128).T),
            "bk": np.ascontiguousarray(bk[sl].reshape(2, 128).T),
            "bv": bv_p,
        })
    return in_maps


def kernel(q, k, v, Wq, bq, Wk, bk, Wv, bv):
    nc = _get_nc()
    in_maps = _shard_inputs(q, k, v, Wq, bq, Wk, bk, Wv, bv)
    res = run_bass_kernel_spmd(nc, in_maps, core_ids=list(range(NCORES)))
    outp = np.empty((B, S, D), np.float32)
    for c in range(NCORES):
        b, g = divmod(c, HPC)
        outp[b, :, E * g:E * (g + 1)] = res.results[c]["out"]
    return outp
